# revision 49
# baseline (speedup 1.0000x reference)
"""ContrastiveLoss Trainium2 kernel (symmetric / triangle scheme).

Contract: kernel(feature, label) -> (loss, mean_pos, mean_neg), matching
reference.reference(). Full inputs in, full outputs out; internally sharded
across 8 NeuronCores.

Strategy: host sorts rows by label (1s first) and L2-normalizes -> z. The
sim matrix exp(2 * z z^T) is symmetric, so each unordered pair is computed
once: global row tile T (of 64 x 128 rows) computes columns [128T, 8192).
Row sums over those columns come from the scalar-engine exp accum; the
transposed contributions (partners i < 128T for each row) are column sums,
computed by streaming the exp'd block (SBUF, bf16) through the PE against a
[label-indicator | ones] stationary matrix, accumulated in PSUM slots packed
by quadrant (16 column-tile slots x [2, 512] at partition offsets 0/32/64/96
across 4 banks). Tiles are paired (T, 63-T) so every core does exactly 260
column-tiles of work; per-core bodies live in a tc.Switch(partition_id, 8).
Host combines row-sum and column-sum partials (float64), applies the log,
and gets mean_pos / mean_neg in closed form from S1/S0.
"""

import sys

sys.path.insert(0, "/opt/trn_rl_repo")

import ml_dtypes
import numpy as np

import concourse.bass as bass
import concourse.mybir as mybir
import concourse.tile as tile
from concourse import bacc
from concourse.bass import ds, ts
from concourse.bass_utils import run_bass_kernel_spmd

B = 8192
D = 128
N_CORES = 8
NT = B // 128                 # 64 global row tiles
TILES_I = 8                   # row tiles per core
CHUNK = 1024                  # psum sim-chunk width (2 banks)
GT = 512                      # global column tile width (psum bank)
NGT = B // GT                 # 16 global column tiles
EPS = 1e-8

FP32 = mybir.dt.float32
BF16 = mybir.dt.bfloat16
AF = mybir.ActivationFunctionType
MAX_SLOTS = 10                # rowsum accum slots per row tile (padded)


def core_tiles(c):
    """Global row-tile indices owned by core c.

    Base assignment pairs (T, 63-T) for equal work; on top of that, one
    heavy high tile is swapped from physical cores 5/6 (measured ~5%
    slower on hardware) to fast cores 1/2.
    """
    low = [4 * c + i for i in range(4)]
    tiles = low + sorted(63 - t for t in low)
    swaps = {1: (59, 43), 5: (43, 59), 2: (55, 39), 6: (39, 55)}
    if c in swaps:
        out_t, in_t = swaps[c]
        tiles = [in_t if t == out_t else t for t in tiles]
        tiles = tiles[:4] + sorted(tiles[4:])
    return tiles


def tile_chunks(T, n1, mini=False):
    """Chunk/segment structure for global row tile T.

    Returns list of chunks; each chunk is a dict with:
      g0: first global column covered by the psum tile (gtile-aligned)
      a, b: computed column range (a >= 128T)
      segs: list of (lo, hi, is_label1) activation ranges (split at n1)
      cols: list of (gt, lo, hi) per-gtile matmul pieces
    """
    diag = 128 * T
    start = GT * (diag // GT)
    chunks = []
    g = start
    nslot = 0
    first = mini
    while g < B:
        # mini: first chunk 512-wide so it needs only one ztc tile and the
        # first exp fires as soon as the first column DMA lands
        step = GT if first else CHUNK
        first = False
        a, b = max(diag, g), min(B, g + step)
        segs = []
        if a < n1 < b:
            segs.append((a, n1, True))
            segs.append((n1, b, False))
        else:
            segs.append((a, b, a < n1))
        cols = []
        for gt in range(g // GT, (b + GT - 1) // GT):
            lo, hi = max(a, gt * GT), min(b, (gt + 1) * GT)
            if lo < hi:
                cols.append((gt, lo, hi))
        chunks.append(dict(g0=g, a=a, b=b, segs=segs, cols=cols,
                           slot0=nslot))
        nslot += len(segs)
        g = b if b > g else g + step
    assert nslot <= MAX_SLOTS, (T, nslot)
    return chunks


def _build_kernel(n1: int):
    nc = bacc.Bacc("TRN2", target_bir_lowering=False, debug=False,
                   num_devices=N_CORES)
    zt = nc.dram_tensor("zt", [D, B], BF16, kind="ExternalInput").ap()
    zlt = nc.dram_tensor("zlt", [D, 128 * TILES_I], BF16,
                         kind="ExternalInput").ap()
    ind = nc.dram_tensor("ind", [128, 2 * TILES_I], BF16,
                         kind="ExternalInput").ap()
    outp = nc.dram_tensor("outp", [128, TILES_I * MAX_SLOTS], FP32,
                          kind="ExternalOutput").ap()
    # colsum slots: [bank, quadrant, 2, 512]; gtile 4b+q -> outc[b, q]
    outc = nc.dram_tensor("outc", [4, 4, 2, GT], FP32,
                          kind="ExternalOutput").ap()

    with tile.TileContext(nc) as tc:
        with (
            tc.tile_pool(name="zc", bufs=1) as zc,
            tc.tile_pool(name="small", bufs=1) as small,
            tc.tile_pool(name="eb", bufs=2) as ebp,
            tc.tile_pool(name="psum", bufs=2, space=bass.MemorySpace.PSUM) as psum,
            tc.tile_pool(name="pcolp", bufs=1, space=bass.MemorySpace.PSUM) as pcolp,
        ):
            # ---- shared (uniform) preamble: DMAs + psum colsum init ----
            # partition_id loads first on the engines whose Switch-arm entry
            # is latency-critical (they read DRAM and take ~1-3us each)
            idx = {}
            for eng in (nc.tensor, nc.scalar, nc.vector):
                idx[eng.engine] = eng.partition_id()

            ztcall = zc.tile([128, NGT, GT], BF16, tag="ztcall")
            ztc = [ztcall[:, k, :] for k in range(NGT)]
            zlT = zc.tile([128, 128 * TILES_I], BF16, tag="zlT")
            indt = small.tile([128, 2 * TILES_I], BF16, tag="indt")
            sacc = small.tile([128, TILES_I, MAX_SLOTS], FP32, tag="sacc")
            # issue schedule: every core's first-needed tiles (its mini
            # chunk uses ztc[c]) land as early as possible on the two
            # DMA-capable queues; the late-consumed tail goes as strips
            zt3 = zt.rearrange("d (k g) -> d k g", g=GT)
            nc.sync.dma_start(zlT[:, 0:128], zlt[:, 0:128])
            nc.gpsimd.dma_start(ztc[1], zt[:, ds(GT, GT)])
            nc.sync.dma_start(ztc[0], zt[:, ds(0, GT)])
            nc.gpsimd.dma_start(ztc[3], zt[:, ds(3 * GT, GT)])
            nc.sync.dma_start(ztc[2], zt[:, ds(2 * GT, GT)])
            nc.gpsimd.dma_start(ztc[5], zt[:, ds(5 * GT, GT)])
            nc.sync.dma_start(ztc[4], zt[:, ds(4 * GT, GT)])
            nc.gpsimd.dma_start(ztc[7], zt[:, ds(7 * GT, GT)])
            nc.sync.dma_start(ztc[6], zt[:, ds(6 * GT, GT)])
            nc.gpsimd.dma_start(ztc[9], zt[:, ds(9 * GT, GT)])
            nc.sync.dma_start(ztc[8], zt[:, ds(8 * GT, GT)])
            nc.gpsimd.dma_start(indt[:], ind)
            nc.sync.dma_start(zlT[:, 128:], zlt[:, 128:])
            nc.gpsimd.dma_start(ztcall[:, 10:13, :], zt3[:, 10:13, :])
            nc.sync.dma_start(ztcall[:, 13:16, :], zt3[:, 13:16, :])
            # scheduler fence: keep the pid load behind the DMA issues so
            # the column tiles start transferring immediately
            tc.no_sync_barrier()
            idx[nc.sync.engine] = nc.sync.partition_id()

            # dummy exp: forces the ACT table load before the Switch
            dum = small.tile([1, 1], FP32, tag="dum")
            nc.vector.memset(dum[:], 0.0)
            nc.scalar.activation(dum[:], dum[:], AF.Exp, scale=2.0)

            pcb = [pcolp.tile([128, GT], FP32, tag=f"pcb{b}", name=f"pcb{b}")
                   for b in range(4)]
            for b in range(4):
                nc.vector.memset(pcb[b][:], 0.0)
            nc.vector.memset(sacc[:], 0.0)

            def cslot(gt):
                return pcb[gt // 4], 32 * (gt % 4)

            cbsb = [small.tile([128, GT], FP32, tag=f"cbsb{b}",
                               name=f"cbsb{b}") for b in range(4)]

            def flush_gtile(g, final=False):
                b, q = g // 4, g % 4
                sl = slice(32 * q, 32 * q + 2)
                nc.vector.tensor_copy(cbsb[b][sl, :], pcb[b][sl, :])
                eng = nc.scalar if (final and g % 2) else nc.sync
                eng.dma_start(outc[b, q], cbsb[b][sl, :])

            # ---- per-core bodies ----
            for c in tc.Switch(idx, N_CORES):
                tlist = core_tiles(c)
                # last tile (processing order) whose colsums touch gtile g
                last_touch = {g: -1 for g in range(NGT)}
                for ti, T in enumerate(tlist):
                    for g in range(NGT):
                        if 128 * (T + 1) < GT * (g + 1):
                            last_touch[g] = max(last_touch[g], ti)
                for g in range(NGT):
                    if last_touch[g] < 0:
                        flush_gtile(g)   # untouched: ship the zeros now
                # software-pipelined emission: colsums lag one chunk
                pending = None  # (ebuf_tile, chunk, T)
                for t, T in enumerate(tlist):
                    for ch in tile_chunks(T, n1, mini=(t == 0)):
                        g0, a, b = ch["g0"], ch["a"], ch["b"]
                        w = b - g0
                        ps = psum.tile([128, CHUNK], FP32, tag="ps")
                        for (gt, lo, hi) in ch["cols"]:
                            nc.tensor.matmul(
                                ps[:, lo - g0:hi - g0],
                                lhsT=zlT[:, ts(t, 128)],
                                rhs=ztc[gt][:, lo - gt * GT:hi - gt * GT],
                                start=True, stop=True)
                        if pending is not None:
                            _emit_colsums(nc, cslot, indt, *pending)
                        eb = ebp.tile([128, CHUNK], BF16, tag="eb")
                        for si, (lo, hi, _l1) in enumerate(ch["segs"]):
                            slot = ch["slot0"] + si
                            nc.scalar.activation(
                                eb[:, lo - g0:hi - g0],
                                ps[:, lo - g0:hi - g0],
                                AF.Exp, scale=2.0,
                                accum_out=sacc[:, t, slot:slot + 1])
                        pending = (eb, ch, t, T)
                    if pending is not None:
                        _emit_colsums(nc, cslot, indt, *pending)
                        pending = None
                    for g in range(NGT):
                        if last_touch[g] == t:
                            flush_gtile(g, final=(t >= TILES_I - 2))
                    eng = nc.scalar if t == TILES_I - 1 else nc.sync
                    eng.dma_start(outp[:, ds(t * MAX_SLOTS, MAX_SLOTS)],
                                  sacc[:, t, :])

    nc.compile()
    return nc


def _emit_colsums(nc, cslot, indt, eb, ch, t, T):
    """Column-sum matmuls for a finished chunk (exp'd values in eb)."""
    g0 = ch["g0"]
    cstart = 128 * (T + 1)   # exclude the diagonal tile
    for (gt, lo, hi) in ch["cols"]:
        lo = max(lo, cstart)
        if lo >= hi:
            continue
        pc, qoff = cslot(gt)
        nc.tensor.matmul(
            pc[qoff:qoff + 2, lo - gt * 512:hi - gt * 512],
            lhsT=indt[:, 2 * t:2 * t + 2],
            rhs=eb[:, lo - g0:hi - g0],
            start=False, stop=True, skip_group_check=True,
            tile_position=(0, qoff))


def plan_slots(n1):
    """Rowsum accumulator slot map; returns {T: [(slot, lo, hi, l1)]}."""
    plan = {}
    for T in range(NT):
        slots = []
        for ch in tile_chunks(T, n1, mini=(T % 4 == 0 and T < 32)):
            for si, (lo, hi, l1) in enumerate(ch["segs"]):
                slots.append((ch["slot0"] + si, lo, hi, l1))
        plan[T] = slots
    return plan


_NC_CACHE = {}


def _get_nc(n1: int = 4083):
    if n1 not in _NC_CACHE:
        _NC_CACHE[n1] = (_build_kernel(n1), plan_slots(n1))
    return _NC_CACHE[n1]


def prepare(feature: np.ndarray, label: np.ndarray):
    """Sort rows by label (1s first), L2-normalize; per-core input maps."""
    feature = np.ascontiguousarray(feature, dtype=np.float32)
    lab = np.asarray(label)
    perm = np.argsort(-lab, kind="stable")
    n1 = int((lab == 1).sum())
    fsort = feature[perm]
    nrm = np.sqrt((fsort.astype(np.float64) ** 2).sum(1))
    z = (fsort / np.maximum(nrm, 1e-12)[:, None].astype(np.float32)).astype(
        np.float32)
    zT = np.ascontiguousarray(z.T.astype(ml_dtypes.bfloat16))
    lsort = lab[perm].astype(np.float64)
    in_maps = []
    for c in range(N_CORES):
        tl = core_tiles(c)
        zl = np.concatenate([zT[:, 128 * T:128 * (T + 1)] for T in tl], axis=1)
        im = np.zeros((128, 2 * TILES_I), np.float32)
        for t, T in enumerate(tl):
            im[:, 2 * t] = lsort[128 * T:128 * (T + 1)]
            im[:, 2 * t + 1] = 1.0
        in_maps.append({
            "zt": zT,
            "zlt": np.ascontiguousarray(zl),
            "ind": im.astype(ml_dtypes.bfloat16),
        })
    return n1, in_maps, z, lsort


def combine(results, n1, plan, z, lsort):
    """Host-side finals from per-core partials (float64)."""
    s1 = np.zeros(B)
    s0 = np.zeros(B)
    for c, r in enumerate(results):
        P = np.asarray(r["outp"], dtype=np.float64).reshape(
            128, TILES_I, MAX_SLOTS)
        for t, T in enumerate(core_tiles(c)):
            rows = slice(128 * T, 128 * (T + 1))
            for (s, lo, hi, l1) in plan[T]:
                if l1:
                    s1[rows] += P[:, t, s]
                else:
                    s0[rows] += P[:, t, s]
        C = np.asarray(r["outc"], dtype=np.float64)  # [bank, quad, 2, 512]
        cs = C.transpose(0, 1, 3, 2).reshape(B, 2)   # gtile-major -> col j
        s1 += cs[:, 0]
        s0 += cs[:, 1] - cs[:, 0]

    sall = s1 + s0
    eii = np.exp(2.0)
    same = np.where(lsort == 1.0, s1, s0)
    num = same - eii
    dennum = sall - eii
    loss = float(np.sum(np.log(dennum) - np.log(num + EPS)) / B)

    zd = z.astype(np.float64)
    S1 = (zd * lsort[:, None]).sum(0)
    S0 = zd.sum(0) - S1
    mean_pos = (S1 @ S1 + S0 @ S0 - B) / (float(B) * B)
    mean_neg = (2.0 * (S1 @ S0)) / (float(B) * B)
    return (np.float32(loss), np.float32(mean_pos), np.float32(mean_neg))


def run_on_hw(feature, label, **kwargs):
    n1, in_maps, z, lsort = prepare(feature, label)
    (nc, plan) = _get_nc(n1)
    res = run_bass_kernel_spmd(nc, in_maps,
                               core_ids=list(range(N_CORES)), **kwargs)
    return combine(res.results, n1, plan, z, lsort), res


def kernel(feature: np.ndarray, label: np.ndarray):
    out, _ = run_on_hw(feature, label)
    return out


# revision 50
# speedup vs baseline: 1.0349x; 1.0349x over previous
"""ContrastiveLoss Trainium2 kernel (symmetric / triangle scheme).

Contract: kernel(feature, label) -> (loss, mean_pos, mean_neg), matching
reference.reference(). Full inputs in, full outputs out; internally sharded
across 8 NeuronCores.

Strategy: host sorts rows by label (1s first) and L2-normalizes -> z. The
sim matrix exp(2 * z z^T) is symmetric, so each unordered pair is computed
once: global row tile T (of 64 x 128 rows) computes columns [128T, 8192).
Row sums over those columns come from the scalar-engine exp accum; the
transposed contributions (partners i < 128T for each row) are column sums,
computed by streaming the exp'd block (SBUF, bf16) through the PE against a
[label-indicator | ones] stationary matrix, accumulated in PSUM slots packed
by quadrant (16 column-tile slots x [2, 512] at partition offsets 0/32/64/96
across 4 banks). Tiles are paired (T, 63-T) so every core does exactly 260
column-tiles of work; per-core bodies live in a tc.Switch(partition_id, 8).
Host combines row-sum and column-sum partials (float64), applies the log,
and gets mean_pos / mean_neg in closed form from S1/S0.
"""

import sys

sys.path.insert(0, "/opt/trn_rl_repo")

import ml_dtypes
import numpy as np

import concourse.bass as bass
import concourse.mybir as mybir
import concourse.tile as tile
from concourse import bacc
from concourse.bass import ds, ts
from concourse.bass_utils import run_bass_kernel_spmd

B = 8192
D = 128
N_CORES = 8
NT = B // 128                 # 64 global row tiles
TILES_I = 8                   # row tiles per core
CHUNK = 1024                  # psum sim-chunk width (2 banks)
GT = 512                      # global column tile width (psum bank)
NGT = B // GT                 # 16 global column tiles
EPS = 1e-8

FP32 = mybir.dt.float32
BF16 = mybir.dt.bfloat16
AF = mybir.ActivationFunctionType
MAX_SLOTS = 10                # rowsum accum slots per row tile (padded)


def core_tiles(c):
    """Global row-tile indices owned by core c.

    Base assignment pairs (T, 63-T) for equal work; on top of that, one
    heavy high tile is swapped from physical cores 5/6 (measured ~5%
    slower on hardware) to fast cores 1/2.
    """
    low = [4 * c + i for i in range(4)]
    tiles = low + sorted(63 - t for t in low)
    swaps = {1: (59, 43), 5: (43, 59), 3: (51, 39), 6: (39, 51)}
    if c in swaps:
        out_t, in_t = swaps[c]
        tiles = [in_t if t == out_t else t for t in tiles]
        tiles = tiles[:4] + sorted(tiles[4:])
    return tiles


def tile_chunks(T, n1, mini=False):
    """Chunk/segment structure for global row tile T.

    Returns list of chunks; each chunk is a dict with:
      g0: first global column covered by the psum tile (gtile-aligned)
      a, b: computed column range (a >= 128T)
      segs: list of (lo, hi, is_label1) activation ranges (split at n1)
      cols: list of (gt, lo, hi) per-gtile matmul pieces
    """
    diag = 128 * T
    start = GT * (diag // GT)
    chunks = []
    g = start
    nslot = 0
    first = mini
    while g < B:
        # mini: first chunk 512-wide so it needs only one ztc tile and the
        # first exp fires as soon as the first column DMA lands
        step = GT if first else CHUNK
        first = False
        a, b = max(diag, g), min(B, g + step)
        segs = []
        if a < n1 < b:
            segs.append((a, n1, True))
            segs.append((n1, b, False))
        else:
            segs.append((a, b, a < n1))
        cols = []
        for gt in range(g // GT, (b + GT - 1) // GT):
            lo, hi = max(a, gt * GT), min(b, (gt + 1) * GT)
            if lo < hi:
                cols.append((gt, lo, hi))
        chunks.append(dict(g0=g, a=a, b=b, segs=segs, cols=cols,
                           slot0=nslot))
        nslot += len(segs)
        g = b if b > g else g + step
    assert nslot <= MAX_SLOTS, (T, nslot)
    return chunks


def _build_kernel(n1: int):
    nc = bacc.Bacc("TRN2", target_bir_lowering=False, debug=False,
                   num_devices=N_CORES)
    zt = nc.dram_tensor("zt", [D, B], BF16, kind="ExternalInput").ap()
    zlt = nc.dram_tensor("zlt", [D, 128 * TILES_I], BF16,
                         kind="ExternalInput").ap()
    ind = nc.dram_tensor("ind", [128, 2 * TILES_I], BF16,
                         kind="ExternalInput").ap()
    outp = nc.dram_tensor("outp", [128, TILES_I * MAX_SLOTS], FP32,
                          kind="ExternalOutput").ap()
    # colsum slots: [bank, quadrant, 2, 512]; gtile 4b+q -> outc[b, q]
    outc = nc.dram_tensor("outc", [4, 4, 2, GT], FP32,
                          kind="ExternalOutput").ap()

    with tile.TileContext(nc) as tc:
        with (
            tc.tile_pool(name="zc", bufs=1) as zc,
            tc.tile_pool(name="small", bufs=1) as small,
            tc.tile_pool(name="eb", bufs=2) as ebp,
            tc.tile_pool(name="psum", bufs=2, space=bass.MemorySpace.PSUM) as psum,
            tc.tile_pool(name="pcolp", bufs=1, space=bass.MemorySpace.PSUM) as pcolp,
        ):
            # ---- shared (uniform) preamble: DMAs + psum colsum init ----
            # partition_id loads first on the engines whose Switch-arm entry
            # is latency-critical (they read DRAM and take ~1-3us each)
            idx = {}
            for eng in (nc.tensor, nc.scalar, nc.vector):
                idx[eng.engine] = eng.partition_id()

            ztcall = zc.tile([128, NGT, GT], BF16, tag="ztcall")
            ztc = [ztcall[:, k, :] for k in range(NGT)]
            zlT = zc.tile([128, 128 * TILES_I], BF16, tag="zlT")
            indt = small.tile([128, 2 * TILES_I], BF16, tag="indt")
            sacc = small.tile([128, TILES_I, MAX_SLOTS], FP32, tag="sacc")
            # issue schedule: every core's first-needed tiles (its mini
            # chunk uses ztc[c]) land as early as possible on the two
            # DMA-capable queues; the late-consumed tail goes as strips
            zt3 = zt.rearrange("d (k g) -> d k g", g=GT)
            nc.sync.dma_start(zlT[:, 0:128], zlt[:, 0:128])
            nc.gpsimd.dma_start(ztc[1], zt[:, ds(GT, GT)])
            nc.sync.dma_start(ztc[0], zt[:, ds(0, GT)])
            nc.gpsimd.dma_start(ztc[3], zt[:, ds(3 * GT, GT)])
            nc.sync.dma_start(ztc[2], zt[:, ds(2 * GT, GT)])
            nc.gpsimd.dma_start(ztc[5], zt[:, ds(5 * GT, GT)])
            nc.sync.dma_start(ztc[4], zt[:, ds(4 * GT, GT)])
            nc.gpsimd.dma_start(ztc[7], zt[:, ds(7 * GT, GT)])
            nc.sync.dma_start(ztc[6], zt[:, ds(6 * GT, GT)])
            nc.gpsimd.dma_start(ztc[9], zt[:, ds(9 * GT, GT)])
            nc.sync.dma_start(ztc[8], zt[:, ds(8 * GT, GT)])
            nc.gpsimd.dma_start(indt[:], ind)
            nc.sync.dma_start(zlT[:, 128:], zlt[:, 128:])
            nc.gpsimd.dma_start(ztcall[:, 10:13, :], zt3[:, 10:13, :])
            nc.sync.dma_start(ztcall[:, 13:16, :], zt3[:, 13:16, :])
            # scheduler fence: keep the pid load behind the DMA issues so
            # the column tiles start transferring immediately
            tc.no_sync_barrier()
            idx[nc.sync.engine] = nc.sync.partition_id()

            # dummy exp: forces the ACT table load before the Switch
            dum = small.tile([1, 1], FP32, tag="dum")
            nc.vector.memset(dum[:], 0.0)
            nc.scalar.activation(dum[:], dum[:], AF.Exp, scale=2.0)

            pcb = [pcolp.tile([128, GT], FP32, tag=f"pcb{b}", name=f"pcb{b}")
                   for b in range(4)]
            for b in range(4):
                nc.vector.memset(pcb[b][:], 0.0)
            nc.vector.memset(sacc[:], 0.0)

            def cslot(gt):
                return pcb[gt // 4], 32 * (gt % 4)

            cbsb = [small.tile([128, GT], FP32, tag=f"cbsb{b}",
                               name=f"cbsb{b}") for b in range(4)]

            def flush_gtile(g, final=False):
                b, q = g // 4, g % 4
                sl = slice(32 * q, 32 * q + 2)
                nc.vector.tensor_copy(cbsb[b][sl, :], pcb[b][sl, :])
                eng = nc.scalar if (final and g % 2) else nc.sync
                eng.dma_start(outc[b, q], cbsb[b][sl, :])

            # ---- per-core bodies ----
            for c in tc.Switch(idx, N_CORES):
                tlist = core_tiles(c)
                # last tile (processing order) whose colsums touch gtile g
                last_touch = {g: -1 for g in range(NGT)}
                for ti, T in enumerate(tlist):
                    for g in range(NGT):
                        if 128 * (T + 1) < GT * (g + 1):
                            last_touch[g] = max(last_touch[g], ti)
                for g in range(NGT):
                    if last_touch[g] < 0:
                        flush_gtile(g)   # untouched: ship the zeros now
                # software-pipelined emission: colsums lag one chunk
                pending = None  # (ebuf_tile, chunk, T)
                for t, T in enumerate(tlist):
                    for ch in tile_chunks(T, n1, mini=(t == 0)):
                        g0, a, b = ch["g0"], ch["a"], ch["b"]
                        w = b - g0
                        ps = psum.tile([128, CHUNK], FP32, tag="ps")
                        for (gt, lo, hi) in ch["cols"]:
                            nc.tensor.matmul(
                                ps[:, lo - g0:hi - g0],
                                lhsT=zlT[:, ts(t, 128)],
                                rhs=ztc[gt][:, lo - gt * GT:hi - gt * GT],
                                start=True, stop=True)
                        if pending is not None:
                            _emit_colsums(nc, cslot, indt, *pending)
                        eb = ebp.tile([128, CHUNK], BF16, tag="eb")
                        for si, (lo, hi, _l1) in enumerate(ch["segs"]):
                            slot = ch["slot0"] + si
                            nc.scalar.activation(
                                eb[:, lo - g0:hi - g0],
                                ps[:, lo - g0:hi - g0],
                                AF.Exp, scale=2.0,
                                accum_out=sacc[:, t, slot:slot + 1])
                        pending = (eb, ch, t, T)
                    if pending is not None:
                        _emit_colsums(nc, cslot, indt, *pending)
                        pending = None
                    for g in range(NGT):
                        if last_touch[g] == t:
                            flush_gtile(g, final=(t >= TILES_I - 2))
                    eng = nc.scalar if t == TILES_I - 1 else nc.sync
                    eng.dma_start(outp[:, ds(t * MAX_SLOTS, MAX_SLOTS)],
                                  sacc[:, t, :])

    nc.compile()
    return nc


def _emit_colsums(nc, cslot, indt, eb, ch, t, T):
    """Column-sum matmuls for a finished chunk (exp'd values in eb)."""
    g0 = ch["g0"]
    cstart = 128 * (T + 1)   # exclude the diagonal tile
    for (gt, lo, hi) in ch["cols"]:
        lo = max(lo, cstart)
        if lo >= hi:
            continue
        pc, qoff = cslot(gt)
        nc.tensor.matmul(
            pc[qoff:qoff + 2, lo - gt * 512:hi - gt * 512],
            lhsT=indt[:, 2 * t:2 * t + 2],
            rhs=eb[:, lo - g0:hi - g0],
            start=False, stop=True, skip_group_check=True,
            tile_position=(0, qoff))


def plan_slots(n1):
    """Rowsum accumulator slot map; returns {T: [(slot, lo, hi, l1)]}."""
    plan = {}
    for T in range(NT):
        slots = []
        for ch in tile_chunks(T, n1, mini=(T % 4 == 0 and T < 32)):
            for si, (lo, hi, l1) in enumerate(ch["segs"]):
                slots.append((ch["slot0"] + si, lo, hi, l1))
        plan[T] = slots
    return plan


_NC_CACHE = {}


def _get_nc(n1: int = 4083):
    if n1 not in _NC_CACHE:
        _NC_CACHE[n1] = (_build_kernel(n1), plan_slots(n1))
    return _NC_CACHE[n1]


def prepare(feature: np.ndarray, label: np.ndarray):
    """Sort rows by label (1s first), L2-normalize; per-core input maps."""
    feature = np.ascontiguousarray(feature, dtype=np.float32)
    lab = np.asarray(label)
    perm = np.argsort(-lab, kind="stable")
    n1 = int((lab == 1).sum())
    fsort = feature[perm]
    nrm = np.sqrt((fsort.astype(np.float64) ** 2).sum(1))
    z = (fsort / np.maximum(nrm, 1e-12)[:, None].astype(np.float32)).astype(
        np.float32)
    zT = np.ascontiguousarray(z.T.astype(ml_dtypes.bfloat16))
    lsort = lab[perm].astype(np.float64)
    in_maps = []
    for c in range(N_CORES):
        tl = core_tiles(c)
        zl = np.concatenate([zT[:, 128 * T:128 * (T + 1)] for T in tl], axis=1)
        im = np.zeros((128, 2 * TILES_I), np.float32)
        for t, T in enumerate(tl):
            im[:, 2 * t] = lsort[128 * T:128 * (T + 1)]
            im[:, 2 * t + 1] = 1.0
        in_maps.append({
            "zt": zT,
            "zlt": np.ascontiguousarray(zl),
            "ind": im.astype(ml_dtypes.bfloat16),
        })
    return n1, in_maps, z, lsort


def combine(results, n1, plan, z, lsort):
    """Host-side finals from per-core partials (float64)."""
    s1 = np.zeros(B)
    s0 = np.zeros(B)
    for c, r in enumerate(results):
        P = np.asarray(r["outp"], dtype=np.float64).reshape(
            128, TILES_I, MAX_SLOTS)
        for t, T in enumerate(core_tiles(c)):
            rows = slice(128 * T, 128 * (T + 1))
            for (s, lo, hi, l1) in plan[T]:
                if l1:
                    s1[rows] += P[:, t, s]
                else:
                    s0[rows] += P[:, t, s]
        C = np.asarray(r["outc"], dtype=np.float64)  # [bank, quad, 2, 512]
        cs = C.transpose(0, 1, 3, 2).reshape(B, 2)   # gtile-major -> col j
        s1 += cs[:, 0]
        s0 += cs[:, 1] - cs[:, 0]

    sall = s1 + s0
    eii = np.exp(2.0)
    same = np.where(lsort == 1.0, s1, s0)
    num = same - eii
    dennum = sall - eii
    loss = float(np.sum(np.log(dennum) - np.log(num + EPS)) / B)

    zd = z.astype(np.float64)
    S1 = (zd * lsort[:, None]).sum(0)
    S0 = zd.sum(0) - S1
    mean_pos = (S1 @ S1 + S0 @ S0 - B) / (float(B) * B)
    mean_neg = (2.0 * (S1 @ S0)) / (float(B) * B)
    return (np.float32(loss), np.float32(mean_pos), np.float32(mean_neg))


def run_on_hw(feature, label, **kwargs):
    n1, in_maps, z, lsort = prepare(feature, label)
    (nc, plan) = _get_nc(n1)
    res = run_bass_kernel_spmd(nc, in_maps,
                               core_ids=list(range(N_CORES)), **kwargs)
    return combine(res.results, n1, plan, z, lsort), res


def kernel(feature: np.ndarray, label: np.ndarray):
    out, _ = run_on_hw(feature, label)
    return out
